# revision 11
# baseline (speedup 1.0000x reference)
"""Trainium2 Bass kernel for the cross-attention layer:

    s   = cosine_sim(em1, em2)          # [B, N, M]
    p   = softmax(s, axis=-1)
    x   = p @ em2                       # [B, N, D]
    out = relu(concat([em1, x]) @ W.T + b)

Sharding: 8 cores, core c = 4*b + i handles batch b, query rows
[i*1024, (i+1)*1024).  em2 is replicated per batch (flash-attention row
sharding).  The score matrix never touches HBM.

Per-core kernel layout choices:
  - QK^T is computed as S^T tiles [m=128, n<=512]: stationary = K^T tile
    (host-pretransposed em2), moving = normalized Q^T (built on-chip via
    PE transposes).  float32r (FP22) matmuls run at 1 cyc/row.
  - key norms are folded into the exp() activation's per-partition scale,
    so raw em2 is used for both K^T and V.
  - V gets a ones-column appended in SBUF; the PV matmul then yields
    [X | rowsum] in one accumulation, and X/rowsum is a per-partition
    scalar multiply.
  - The final FC runs off two PSUM accumulations: A = Qnorm^T.T @ W1^T
    (rescaled by per-row 1/||q|| afterwards, avoiding a second transpose
    of raw em1) and B = Xnorm^T.T @ W2^T + bias (ones-row matmul).
"""

import sys

if "/opt/trn_rl_repo" not in sys.path:
    sys.path.insert(0, "/opt/trn_rl_repo")

from contextlib import ExitStack

import numpy as np

import concourse.bass as bass
import concourse.mybir as mybir
import concourse.tile as tile
from concourse import bacc
from concourse.bass_utils import run_bass_kernel_spmd
from concourse.masks import make_identity

B, N, M, D = 2, 4096, 4096, 256
NSH = N // 4          # query rows per core
P = 128
NT = NSH // P         # 8 query tiles per core
MT = M // P           # 32 key tiles
OUT = 512
EPS = 1e-6
F32 = mybir.dt.float32
F32R = mybir.dt.float32r
ACTF = mybir.ActivationFunctionType
ALU = mybir.AluOpType

NBLK = 512            # query columns per S^T block
NBLKS = NSH // NBLK   # 2


def r32(ap):
    return ap.bitcast(F32R)


def build_nc(debug=False):
    nc = bacc.Bacc("TRN2", target_bir_lowering=False)
    q_d = nc.declare_dram_parameter("q", [NSH, D], F32, isOutput=False)
    kt_d = nc.declare_dram_parameter("kt", [D, M], F32, isOutput=False)
    v_d = nc.declare_dram_parameter("v", [M, D], F32, isOutput=False)
    wt_d = nc.declare_dram_parameter("wt", [2 * D, OUT], F32, isOutput=False)
    b_d = nc.declare_dram_parameter("bias", [1, OUT], F32, isOutput=False)
    out_d = nc.declare_dram_parameter("out", [NSH, OUT], F32, isOutput=True)
    if debug:
        dbg_qt = nc.declare_dram_parameter("dbg_qt", [P, 2, NSH], F32, isOutput=True)
        dbg_rk = nc.declare_dram_parameter("dbg_rk", [P, MT], F32, isOutput=True)
        dbg_rq = nc.declare_dram_parameter("dbg_rq", [P, NT], F32, isOutput=True)
        dbg_pt = nc.declare_dram_parameter("dbg_pt", [P, NBLK], F32, isOutput=True)
        dbg_xn = nc.declare_dram_parameter("dbg_xn", [P, D], F32, isOutput=True)
        dbg_ri = nc.declare_dram_parameter("dbg_ri", [P, NT], F32, isOutput=True)

    with ExitStack() as ctx:
        tc = ctx.enter_context(tile.TileContext(nc))
        sb = ctx.enter_context(tc.tile_pool(name="sb", bufs=1))
        sbw = ctx.enter_context(tc.tile_pool(name="sbw", bufs=3))
        psA = ctx.enter_context(tc.tile_pool(name="psA", bufs=2, space="PSUM"))
        psX = ctx.enter_context(tc.tile_pool(name="psX", bufs=4, space="PSUM"))
        psF = ctx.enter_context(tc.tile_pool(name="psF", bufs=2, space="PSUM"))

        # ---- persistent SBUF buffers ----
        # K^T and V are split into 4 chunk tiles each so that consumers
        # wait on exactly one DMA completion lane (HW sync-wait limit).
        ktc = [
            sb.tile([P, 2, M // 4], F32R, tag=f"ktc{g}", name=f"ktc{g}")
            for g in range(4)
        ]
        # V' has D+2 columns: col D = ones (rowsum trick), col D+1 = zeros
        # (fp32r matmuls need an even moving free size).
        vc = [
            sb.tile([P, MT // 4, D + 2], F32R, tag=f"vc{g}", name=f"vc{g}")
            for g in range(4)
        ]
        qbuf = sb.tile([P, NT, D], F32, tag="qbuf")         # raw Q, natural
        qtbuf = sb.tile([P, 2, NSH], F32R, tag="qtbuf")      # normalized Q^T
        wtbuf = sb.tile([P, 4, OUT], F32R, tag="wtbuf")      # W^T feature tiles
        bbuf = sb.tile([1, OUT], F32R, tag="bbuf")           # bias row
        hbuf = sb.tile([P, NT, OUT], F32, tag="hbuf")       # output staging
        ident = sb.tile([P, P], F32, tag="ident")
        ones_row = sb.tile([1, P], F32R, tag="ones_row")
        n2k = sb.tile([P, MT], F32, tag="n2k")
        nk = sb.tile([P, MT], F32, tag="nk")
        rk = sb.tile([P, MT], F32, tag="rk")                # 1/||k|| per key row
        n2q = sb.tile([P, NT], F32, tag="n2q")
        nq = sb.tile([P, NT], F32, tag="nq")
        rq = sb.tile([P, NT], F32, tag="rq")                # 1/||q|| per query row
        rinv = sb.tile([P, NT], F32, tag="rinv")            # 1/rowsum per query row

        make_identity(nc, ident)
        # f32r memsets are not a legal ISA encoding; memset f32 staging
        # tiles and cast-copy into the f32r buffers instead.
        ones_f32 = sb.tile([1, P], F32, tag="ones_f32")
        nc.vector.memset(ones_f32, 1.0)
        nc.vector.tensor_copy(out=ones_row, in_=ones_f32)
        vcol_f32 = sb.tile([P, MT // 4, 2], F32, tag="vcol_f32")
        nc.vector.memset(vcol_f32, 0.0)
        nc.vector.memset(vcol_f32[:, :, 0:1], 1.0)
        for g in range(4):
            nc.vector.tensor_copy(out=vc[g][:, :, D : D + 2], in_=vcol_f32)

        # ---- DMAs (big, chunked so compute can start early) ----
        kt_r = kt_d[:].rearrange("(do p) m -> p do m", p=P)
        for g in range(4):
            s = slice(g * (M // 4), (g + 1) * (M // 4))
            nc.sync.dma_start(ktc[g][:], kt_r[:, :, s].bitcast(F32R))
        v_r = v_d[:].rearrange("(mo p) d -> p mo d", p=P)
        for g in range(4):
            s = slice(g * (MT // 4), (g + 1) * (MT // 4))
            nc.sync.dma_start(vc[g][:, :, 0:D], v_r[:, s, :].bitcast(F32R))
        nc.sync.dma_start(qbuf[:], q_d[:].rearrange("(no p) d -> p no d", p=P))
        nc.sync.dma_start(wtbuf[:], wt_d[:].rearrange("(fo p) o -> p fo o", p=P).bitcast(F32R))
        nc.sync.dma_start(bbuf[:], b_d[:].bitcast(F32R))

        # ---- key norms: rk = 1/sqrt(max(sum(k^2), eps)), chunked ----
        for mg in range(4):
            cs = slice(mg * (MT // 4), (mg + 1) * (MT // 4))
            for m in range(mg * (MT // 4), (mg + 1) * (MT // 4)):
                sq = sbw.tile([P, D], F32, tag="sqs")
                nc.scalar.activation(
                    sq, vc[m // 8][:, m % 8, 0:D].bitcast(F32), ACTF.Square,
                    accum_out=n2k[:, m : m + 1],
                )
            nc.vector.tensor_scalar_max(n2k[:, cs], n2k[:, cs], EPS)
            nc.scalar.sqrt(nk[:, cs], n2k[:, cs])
            nc.vector.reciprocal(rk[:, cs], nk[:, cs])

        # ---- query norms + normalized Q^T ----
        for t in range(NT):
            sq = sbw.tile([P, D], F32, tag="sqs")
            nc.scalar.activation(
                sq, qbuf[:, t, :], ACTF.Square, accum_out=n2q[:, t : t + 1]
            )
        nc.vector.tensor_scalar_max(n2q[:], n2q[:], EPS)
        nc.scalar.sqrt(nq[:], n2q[:])
        nc.vector.reciprocal(rq[:], nq[:])
        for t in range(NT):
            qn = sbw.tile([P, D], F32, tag="qn")
            nc.vector.tensor_scalar_mul(qn, qbuf[:, t, :], rq[:, t : t + 1])
            for dt in range(2):
                tp = psA.tile([P, P], F32, tag="sp")
                nc.tensor.transpose(tp, qn[:, dt * P : (dt + 1) * P], ident)
                nc.vector.tensor_copy(
                    out=qtbuf[:, dt, t * P : (t + 1) * P], in_=tp
                )

        # ---- main flash-attention loop ----
        for nb in range(NBLKS):
            ncols = slice(nb * NBLK, (nb + 1) * NBLK)
            xps = [
                psX.tile([P, D + 2], F32, tag="xp", name=f"xp_{nb}_{j}")
                for j in range(4)
            ]
            pts = {}
            for m in range(MT + 1):
                if m < MT:
                    sp = psA.tile([P, NBLK], F32, tag="sp")
                    ktg = ktc[m // 8]
                    ms = slice((m % 8) * P, (m % 8 + 1) * P)
                    nc.tensor.matmul(
                        sp, ktg[:, 0, ms], qtbuf[:, 0, ncols],
                        start=True, stop=False,
                    )
                    nc.tensor.matmul(
                        sp, ktg[:, 1, ms], qtbuf[:, 1, ncols],
                        start=False, stop=True,
                    )
                    pt = sbw.tile([P, NBLK], F32R, tag="pt")
                    nc.scalar.activation(pt, sp, ACTF.Exp, scale=rk[:, m : m + 1])
                    pts[m] = pt
                    if debug and nb == 0 and m == 0:
                        nc.sync.dma_start(dbg_pt[:], pt[:].bitcast(F32))
                if m >= 1:
                    mm = m - 1
                    pt = pts.pop(mm)
                    for j in range(4):
                        nc.tensor.matmul(
                            xps[j],
                            pt[:, j * P : (j + 1) * P],
                            vc[mm // 8][:, mm % 8, :],
                            start=(mm == 0), stop=(mm == MT - 1),
                        )

            # ---- per-tile epilogue: normalize X, transpose, FC, relu ----
            for j in range(4):
                t = nb * 4 + j
                ts_ = slice(t * P, (t + 1) * P)
                nc.vector.reciprocal(rinv[:, t : t + 1], xps[j][:, D : D + 1])
                xn = sbw.tile([P, D], F32, tag="xn")
                nc.vector.tensor_scalar_mul(xn, xps[j][:, 0:D], rinv[:, t : t + 1])
                if debug and t == 0:
                    nc.sync.dma_start(dbg_xn[:], xn[:])
                xnt = sbw.tile([P, 2, P], F32R, tag="xnt")
                for dt in range(2):
                    tp = psA.tile([P, P], F32, tag="sp")
                    nc.tensor.transpose(tp, xn[:, dt * P : (dt + 1) * P], ident)
                    nc.vector.tensor_copy(out=xnt[:, dt, :], in_=tp)

                ap_ = psF.tile([P, OUT], F32, tag="fc")
                bp_ = psF.tile([P, OUT], F32, tag="fc")
                nc.tensor.matmul(
                    ap_, qtbuf[:, 0, ts_], wtbuf[:, 0, :],
                    start=True, stop=False,
                )
                nc.tensor.matmul(
                    ap_, qtbuf[:, 1, ts_], wtbuf[:, 1, :],
                    start=False, stop=True,
                )
                nc.tensor.matmul(
                    bp_, xnt[:, 0, :], wtbuf[:, 2, :],
                    start=True, stop=False,
                )
                nc.tensor.matmul(
                    bp_, xnt[:, 1, :], wtbuf[:, 3, :],
                    start=False, stop=False,
                )
                nc.tensor.matmul(
                    bp_, ones_row, bbuf, start=False, stop=True,
                )
                t1 = sbw.tile([P, OUT], F32, tag="t1")
                nc.vector.tensor_scalar_mul(t1, ap_, nq[:, t : t + 1])
                nc.vector.tensor_add(out=hbuf[:, t, :], in0=t1, in1=bp_)
                nc.vector.tensor_scalar_max(hbuf[:, t, :], hbuf[:, t, :], 0.0)

        nc.sync.dma_start(out_d[:].rearrange("(no p) o -> p no o", p=P), hbuf[:])
        if debug:
            nc.sync.dma_start(dbg_qt[:], qtbuf[:])
            nc.sync.dma_start(dbg_rk[:], rk[:])
            nc.sync.dma_start(dbg_rq[:], rq[:])
            nc.sync.dma_start(dbg_ri[:], rinv[:])

    nc.compile()
    return nc


_NC = None


def _get_nc():
    global _NC
    if _NC is None:
        _NC = build_nc()
    return _NC


def _run(inputs, trace=False):
    em1 = np.asarray(inputs["em1"], dtype=np.float32)
    em2 = np.asarray(inputs["em2"], dtype=np.float32)
    W = np.asarray(inputs["W"], dtype=np.float32)
    b = np.asarray(inputs["b"], dtype=np.float32)

    wt = np.ascontiguousarray(W.T)
    brow = np.ascontiguousarray(b[None, :])
    in_maps = []
    for c in range(8):
        bi, qi = c // 4, c % 4
        in_maps.append(
            {
                "q": np.ascontiguousarray(em1[bi, qi * NSH : (qi + 1) * NSH]),
                "kt": np.ascontiguousarray(em2[bi].T),
                "v": np.ascontiguousarray(em2[bi]),
                "wt": wt,
                "bias": brow,
            }
        )

    res = run_bass_kernel_spmd(_get_nc(), in_maps, core_ids=list(range(8)), trace=trace)
    out = np.empty((B, N, OUT), dtype=np.float32)
    for c in range(8):
        bi, qi = c // 4, c % 4
        out[bi, qi * NSH : (qi + 1) * NSH] = res.results[c]["out"]
    return out, res


def kernel(**inputs) -> np.ndarray:
    out, _ = _run(inputs, trace=False)
    return out


# revision 15
# speedup vs baseline: 1.0696x; 1.0696x over previous
"""Trainium2 Bass kernel for the cross-attention layer:

    s   = cosine_sim(em1, em2)          # [B, N, M]
    p   = softmax(s, axis=-1)
    x   = p @ em2                       # [B, N, D]
    out = relu(concat([em1, x]) @ W.T + b)

Sharding: 8 cores, core c = 4*b + i handles batch b, query rows
[i*1024, (i+1)*1024).  em2 is replicated per batch (flash-attention row
sharding).  The score matrix never touches HBM.

Per-core layout choices:
  - All matmul operands are bf16 (separate, pipelined LDWEIGHTS; fp32/
    fp32r matmuls self-load weights and serialize ~176ns per matmul).
    Accumulation stays fp32 in PSUM.
  - QK^T is computed as S^T tiles [m=128, n<=512]: stationary = K^T tile
    (host-pretransposed em2 in bf16), moving = normalized Q^T (built
    on-chip via PE transposes).
  - key norms are folded into the exp() activation's per-partition
    scale, so raw em2 serves as both K^T and V; exp writes bf16 P^T
    tiles that feed the PV matmul directly as stationary weights.
  - V gets a ones-column appended in SBUF; the PV matmul then yields
    [X | rowsum] in one accumulation and X/rowsum is a per-partition
    scalar multiply.
  - The final FC runs off two PSUM accumulations: A = Qnorm^T.T @ W1^T
    (rescaled by per-row ||q|| afterwards, avoiding a transpose of raw
    em1) and B = Xnorm^T.T @ W2^T + bias (ones-row matmul).
  - Norm square-reductions run on VectorE (tensor_tensor_reduce), not
    ScalarE: ScalarE is saturated by the 64 exp() tiles.
"""

import sys

if "/opt/trn_rl_repo" not in sys.path:
    sys.path.insert(0, "/opt/trn_rl_repo")

from contextlib import ExitStack

import numpy as np

import concourse.bass as bass
import concourse.mybir as mybir
import concourse.tile as tile
from concourse import bacc
from concourse.bass_utils import run_bass_kernel_spmd
from concourse.masks import make_identity

B, N, M, D = 2, 4096, 4096, 256
NSH = N // 4          # query rows per core
P = 128
NT = NSH // P         # 8 query tiles per core
MT = M // P           # 32 key tiles
OUT = 512
EPS = 1e-6
F32 = mybir.dt.float32
F32R = mybir.dt.float32r
BF16 = mybir.dt.bfloat16
ACTF = mybir.ActivationFunctionType
ALU = mybir.AluOpType
NPBF16 = mybir.dt.np(BF16)

NBLK = 512            # query columns per S^T block
NBLKS = NSH // NBLK   # 2
VW = D + 2            # V' width: ones col at D, zero pad at D+1


def build_nc(debug=False):
    nc = bacc.Bacc("TRN2", target_bir_lowering=False)
    q_d = nc.declare_dram_parameter("q", [NSH, D], F32, isOutput=False)
    kt_d = nc.declare_dram_parameter("kt", [D, M], F32, isOutput=False)
    v_d = nc.declare_dram_parameter("v", [M, D], F32, isOutput=False)
    wt_d = nc.declare_dram_parameter("wt", [2 * D, OUT], F32, isOutput=False)
    b_d = nc.declare_dram_parameter("bias", [1, OUT], F32, isOutput=False)
    out_d = nc.declare_dram_parameter("out", [NSH, OUT], F32, isOutput=True)
    if debug:
        dbg_qt = nc.declare_dram_parameter("dbg_qt", [P, 2, NSH], F32, isOutput=True)
        dbg_rk = nc.declare_dram_parameter("dbg_rk", [P, MT], F32, isOutput=True)
        dbg_rq = nc.declare_dram_parameter("dbg_rq", [P, NT], F32, isOutput=True)
        dbg_pt = nc.declare_dram_parameter("dbg_pt", [P, NBLK], F32, isOutput=True)
        dbg_xn = nc.declare_dram_parameter("dbg_xn", [P, D], F32, isOutput=True)
        dbg_ri = nc.declare_dram_parameter("dbg_ri", [P, NT], F32, isOutput=True)

    with ExitStack() as ctx:
        tc = ctx.enter_context(tile.TileContext(nc))
        sb = ctx.enter_context(tc.tile_pool(name="sb", bufs=1))
        sbw = ctx.enter_context(tc.tile_pool(name="sbw", bufs=3))
        psA = ctx.enter_context(tc.tile_pool(name="psA", bufs=2, space="PSUM"))
        psX = ctx.enter_context(tc.tile_pool(name="psX", bufs=4, space="PSUM"))
        psF = ctx.enter_context(tc.tile_pool(name="psF", bufs=2, space="PSUM"))

        # ---- persistent SBUF buffers ----
        qbuf = sb.tile([P, NT, D], F32, tag="qbuf")         # raw Q, natural
        ktc = [
            sb.tile([P, 2, M // 4], F32R, tag=f"ktc{g}", name=f"ktc{g}")
            for g in range(4)
        ]
        vc = [
            sb.tile([P, MT // 4, VW], F32R, tag=f"vc{g}", name=f"vc{g}")
            for g in range(4)
        ]
        qtbuf = sb.tile([P, 2, NSH], F32R, tag="qtbuf")      # normalized Q^T (QK moving)
        qt32 = sb.tile([P, 2, NSH], F32R, tag="qt32")        # normalized Q^T (FC stationary)
        wtbuf = sb.tile([P, 4, OUT], F32R, tag="wtbuf")      # W^T feature tiles
        bbuf = sb.tile([1, OUT], F32R, tag="bbuf")           # bias row
        hbuf = sb.tile([P, NT, OUT], F32, tag="hbuf")        # output staging
        ident = sb.tile([P, P], F32, tag="ident")
        ones_row = sb.tile([1, P], F32R, tag="ones_row")
        ones_f32 = sb.tile([1, P], F32, tag="ones_f32")
        n2k = sb.tile([P, MT], F32, tag="n2k")
        nk = sb.tile([P, MT], F32, tag="nk")
        rk = sb.tile([P, MT], F32, tag="rk")                # 1/||k|| per key row
        n2q = sb.tile([P, NT], F32, tag="n2q")
        nq = sb.tile([P, NT], F32, tag="nq")                # ||q|| per query row
        rq = sb.tile([P, NT], F32, tag="rq")                # 1/||q||
        rinv = sb.tile([P, NT], F32, tag="rinv")            # 1/rowsum

        make_identity(nc, ident)
        nc.vector.memset(ones_f32, 1.0)
        nc.vector.tensor_copy(out=ones_row, in_=ones_f32)
        vcol_f32 = sb.tile([P, MT // 4, 2], F32, tag="vcol_f32")
        nc.vector.memset(vcol_f32, 0.0)
        nc.vector.memset(vcol_f32[:, :, 0:1], 1.0)
        for g in range(4):
            nc.vector.tensor_copy(out=vc[g][:, :, D : D + 2], in_=vcol_f32)

        # ---- DMAs; q first (it gates the Q-norm -> QK chain) ----
        nc.sync.dma_start(qbuf[:], q_d[:].rearrange("(no p) d -> p no d", p=P))
        kt_r = kt_d[:].rearrange("(do p) m -> p do m", p=P)
        v_r = v_d[:].rearrange("(mo p) d -> p mo d", p=P)
        for g in range(4):
            s = slice(g * (M // 4), (g + 1) * (M // 4))
            nc.sync.dma_start(ktc[g][:], kt_r[:, :, s].bitcast(F32R))
            sv = slice(g * (MT // 4), (g + 1) * (MT // 4))
            nc.sync.dma_start(vc[g][:, :, 0:D], v_r[:, sv, :].bitcast(F32R))
        nc.sync.dma_start(wtbuf[:], wt_d[:].rearrange("(fo p) o -> p fo o", p=P).bitcast(F32R))
        nc.sync.dma_start(bbuf[:], b_d[:].bitcast(F32R))

        # ---- query norms + normalized Q^T (the critical-path chain) ----
        for t in range(NT):
            sq = sbw.tile([P, D], F32, tag="sqs")
            nc.scalar.activation(
                sq, qbuf[:, t, :], ACTF.Square, accum_out=n2q[:, t : t + 1]
            )
        nc.vector.tensor_scalar_max(n2q[:], n2q[:], EPS)
        nc.scalar.sqrt(nq[:], n2q[:])
        nc.vector.reciprocal(rq[:], nq[:])
        for t in range(NT):
            qn = sbw.tile([P, D], F32, tag="qn")
            nc.vector.tensor_scalar_mul(qn, qbuf[:, t, :], rq[:, t : t + 1])
            for dt in range(2):
                tp = psA.tile([P, P], F32, tag="sp")
                nc.tensor.transpose(tp, qn[:, dt * P : (dt + 1) * P], ident)
                nc.vector.tensor_copy(
                    out=qtbuf[:, dt, t * P : (t + 1) * P], in_=tp
                )
                nc.vector.tensor_copy(
                    out=qt32[:, dt, t * P : (t + 1) * P], in_=tp
                )

        # ---- key norms: rk = 1/sqrt(max(sum(k^2), eps)), per chunk ----
        for g in range(4):
            cs = slice(g * (MT // 4), (g + 1) * (MT // 4))
            for mm in range(MT // 4):
                m = g * (MT // 4) + mm
                sq = sbw.tile([P, D], F32, tag="sqk")
                nc.scalar.activation(
                    sq, vc[g][:, mm, 0:D].bitcast(F32), ACTF.Square,
                    accum_out=n2k[:, m : m + 1],
                )
            nc.vector.tensor_scalar_max(n2k[:, cs], n2k[:, cs], EPS)
            nc.scalar.sqrt(nk[:, cs], n2k[:, cs])
            nc.vector.reciprocal(rk[:, cs], nk[:, cs])

        # ---- main flash-attention loop ----
        for nb in range(NBLKS):
            ncols = slice(nb * NBLK, (nb + 1) * NBLK)
            xps = [
                psX.tile([P, VW], F32, tag="xp", name=f"xp_{nb}_{j}")
                for j in range(4)
            ]
            pts = {}
            for m in range(MT + 1):
                if m < MT:
                    sp = psA.tile([P, NBLK], F32, tag="sp")
                    ktg = ktc[m // 8]
                    ms = slice((m % 8) * P, (m % 8 + 1) * P)
                    nc.tensor.matmul(
                        sp, ktg[:, 0, ms], qtbuf[:, 0, ncols],
                        start=True, stop=False,
                    )
                    nc.tensor.matmul(
                        sp, ktg[:, 1, ms], qtbuf[:, 1, ncols],
                        start=False, stop=True,
                    )
                    pt = sbw.tile([P, NBLK], F32R, tag="pt")
                    nc.scalar.activation(pt, sp, ACTF.Exp, scale=rk[:, m : m + 1])
                    pts[m] = pt
                    if debug and nb == 0 and m == 0:
                        nc.sync.dma_start(dbg_pt[:], pt[:].bitcast(F32))
                if m >= 1:
                    mm = m - 1
                    pt = pts.pop(mm)
                    for j in range(4):
                        nc.tensor.matmul(
                            xps[j],
                            pt[:, j * P : (j + 1) * P],
                            vc[mm // 8][:, mm % 8, :],
                            start=(mm == 0), stop=(mm == MT - 1),
                        )

            # ---- per-tile epilogue: normalize X, transpose, FC, relu ----
            for j in range(4):
                t = nb * 4 + j
                ts_ = slice(t * P, (t + 1) * P)
                nc.vector.reciprocal(rinv[:, t : t + 1], xps[j][:, D : D + 1])
                xn = sbw.tile([P, D], F32, tag="xn")
                nc.vector.tensor_scalar_mul(xn, xps[j][:, 0:D], rinv[:, t : t + 1])
                if debug and t == 0:
                    nc.sync.dma_start(dbg_xn[:], xn[:])
                xnt = sbw.tile([P, 2, P], F32R, tag="xnt")
                for dt in range(2):
                    tp = psA.tile([P, P], F32, tag="sp")
                    nc.tensor.transpose(tp, xn[:, dt * P : (dt + 1) * P], ident)
                    nc.vector.tensor_copy(out=xnt[:, dt, :], in_=tp)

                ap_ = psF.tile([P, OUT], F32, tag="fc")
                bp_ = psF.tile([P, OUT], F32, tag="fc")
                nc.tensor.matmul(
                    ap_, qt32[:, 0, ts_], wtbuf[:, 0, :],
                    start=True, stop=False,
                )
                nc.tensor.matmul(
                    ap_, qt32[:, 1, ts_], wtbuf[:, 1, :],
                    start=False, stop=True,
                )
                nc.tensor.matmul(
                    bp_, xnt[:, 0, :], wtbuf[:, 2, :],
                    start=True, stop=False,
                )
                nc.tensor.matmul(
                    bp_, xnt[:, 1, :], wtbuf[:, 3, :],
                    start=False, stop=False,
                )
                nc.tensor.matmul(
                    bp_, ones_row, bbuf, start=False, stop=True,
                )
                t1 = sbw.tile([P, OUT], F32, tag="t1")
                nc.vector.tensor_scalar_mul(t1, ap_, nq[:, t : t + 1])
                nc.vector.tensor_add(out=hbuf[:, t, :], in0=t1, in1=bp_)
                nc.vector.tensor_scalar_max(hbuf[:, t, :], hbuf[:, t, :], 0.0)

        nc.sync.dma_start(out_d[:].rearrange("(no p) o -> p no o", p=P), hbuf[:])
        if debug:
            nc.sync.dma_start(dbg_qt[:], qtbuf[:].bitcast(F32))
            nc.sync.dma_start(dbg_rk[:], rk[:])
            nc.sync.dma_start(dbg_rq[:], rq[:])
            nc.sync.dma_start(dbg_ri[:], rinv[:])

    nc.compile()
    return nc


_NC = None


def _get_nc():
    global _NC
    if _NC is None:
        _NC = build_nc()
    return _NC


def _run(inputs, trace=False):
    em1 = np.asarray(inputs["em1"], dtype=np.float32)
    em2 = np.asarray(inputs["em2"], dtype=np.float32)
    W = np.asarray(inputs["W"], dtype=np.float32)
    b = np.asarray(inputs["b"], dtype=np.float32)

    wt = np.ascontiguousarray(W.T)
    brow = np.ascontiguousarray(b[None, :])
    kts = [np.ascontiguousarray(em2[bi].T) for bi in range(B)]
    vs = [np.ascontiguousarray(em2[bi]) for bi in range(B)]
    in_maps = []
    for c in range(8):
        bi, qi = c // 4, c % 4
        in_maps.append(
            {
                "q": np.ascontiguousarray(em1[bi, qi * NSH : (qi + 1) * NSH]),
                "kt": kts[bi],
                "v": vs[bi],
                "wt": wt,
                "bias": brow,
            }
        )

    res = run_bass_kernel_spmd(_get_nc(), in_maps, core_ids=list(range(8)), trace=trace)
    out = np.empty((B, N, OUT), dtype=np.float32)
    for c in range(8):
        bi, qi = c // 4, c % 4
        out[bi, qi * NSH : (qi + 1) * NSH] = res.results[c]["out"]
    return out, res


def kernel(**inputs) -> np.ndarray:
    out, _ = _run(inputs, trace=False)
    return out


# revision 16
# speedup vs baseline: 1.1817x; 1.1048x over previous
"""Trainium2 Bass kernel for the cross-attention layer:

    s   = cosine_sim(em1, em2)          # [B, N, M]
    p   = softmax(s, axis=-1)
    x   = p @ em2                       # [B, N, D]
    out = relu(concat([em1, x]) @ W.T + b)

Sharding: 8 cores, core c = 4*b + i handles batch b, query rows
[i*1024, (i+1)*1024).  em2 is replicated per batch (flash-attention row
sharding).  The score matrix never touches HBM.

Per-core layout choices:
  - All matmul operands are bf16 (separate, pipelined LDWEIGHTS; fp32/
    fp32r matmuls self-load weights and serialize ~176ns per matmul).
    Accumulation stays fp32 in PSUM.
  - QK^T is computed as S^T tiles [m=128, n<=512]: stationary = K^T tile
    (host-pretransposed em2 in bf16), moving = normalized Q^T (built
    on-chip via PE transposes).
  - key norms are folded into the exp() activation's per-partition
    scale, so raw em2 serves as both K^T and V; exp writes bf16 P^T
    tiles that feed the PV matmul directly as stationary weights.
  - V gets a ones-column appended in SBUF; the PV matmul then yields
    [X | rowsum] in one accumulation and X/rowsum is a per-partition
    scalar multiply.
  - The final FC runs off two PSUM accumulations: A = Qnorm^T.T @ W1^T
    (rescaled by per-row ||q|| afterwards, avoiding a transpose of raw
    em1) and B = Xnorm^T.T @ W2^T + bias (ones-row matmul).
  - Norm square-reductions run on VectorE (tensor_tensor_reduce), not
    ScalarE: ScalarE is saturated by the 64 exp() tiles.
"""

import sys

if "/opt/trn_rl_repo" not in sys.path:
    sys.path.insert(0, "/opt/trn_rl_repo")

from contextlib import ExitStack

import numpy as np

import concourse.bass as bass
import concourse.mybir as mybir
import concourse.tile as tile
from concourse import bacc
from concourse.bass_utils import run_bass_kernel_spmd
from concourse.masks import make_identity

B, N, M, D = 2, 4096, 4096, 256
NSH = N // 4          # query rows per core
P = 128
NT = NSH // P         # 8 query tiles per core
MT = M // P           # 32 key tiles
OUT = 512
EPS = 1e-6
F32 = mybir.dt.float32
F32R = mybir.dt.float32r
BF16 = mybir.dt.bfloat16
ACTF = mybir.ActivationFunctionType
ALU = mybir.AluOpType
NPBF16 = mybir.dt.np(BF16)

NBLK = 512            # query columns per S^T block
NBLKS = NSH // NBLK   # 2
VW = D + 2            # V' width: ones col at D, zero pad at D+1


def build_nc(debug=False):
    nc = bacc.Bacc("TRN2", target_bir_lowering=False)
    q_d = nc.declare_dram_parameter("q", [NSH, D], F32, isOutput=False)
    kt_d = nc.declare_dram_parameter("kt", [D, M], BF16, isOutput=False)
    v_d = nc.declare_dram_parameter("v", [M, D], BF16, isOutput=False)
    wt_d = nc.declare_dram_parameter("wt", [2 * D, OUT], F32, isOutput=False)
    b_d = nc.declare_dram_parameter("bias", [1, OUT], F32, isOutput=False)
    out_d = nc.declare_dram_parameter("out", [NSH, OUT], F32, isOutput=True)
    if debug:
        dbg_qt = nc.declare_dram_parameter("dbg_qt", [P, 2, NSH], BF16, isOutput=True)
        dbg_rk = nc.declare_dram_parameter("dbg_rk", [P, MT], F32, isOutput=True)
        dbg_rq = nc.declare_dram_parameter("dbg_rq", [P, NT], F32, isOutput=True)
        dbg_pt = nc.declare_dram_parameter("dbg_pt", [P, NBLK], BF16, isOutput=True)
        dbg_xn = nc.declare_dram_parameter("dbg_xn", [P, D], F32, isOutput=True)
        dbg_ri = nc.declare_dram_parameter("dbg_ri", [P, NT], F32, isOutput=True)

    with ExitStack() as ctx:
        tc = ctx.enter_context(tile.TileContext(nc))
        sb = ctx.enter_context(tc.tile_pool(name="sb", bufs=1))
        sbw = ctx.enter_context(tc.tile_pool(name="sbw", bufs=3))
        psA = ctx.enter_context(tc.tile_pool(name="psA", bufs=2, space="PSUM"))
        psX = ctx.enter_context(tc.tile_pool(name="psX", bufs=4, space="PSUM"))
        psF = ctx.enter_context(tc.tile_pool(name="psF", bufs=2, space="PSUM"))

        # ---- persistent SBUF buffers ----
        qbuf = sb.tile([P, NT, D], F32, tag="qbuf")         # raw Q, natural
        ktc = [
            sb.tile([P, 2, M // 4], BF16, tag=f"ktc{g}", name=f"ktc{g}")
            for g in range(4)
        ]
        vc = [
            sb.tile([P, MT // 4, VW], BF16, tag=f"vc{g}", name=f"vc{g}")
            for g in range(4)
        ]
        qtbuf = sb.tile([P, 2, NSH], BF16, tag="qtbuf")      # normalized Q^T (QK moving)
        qt32 = sb.tile([P, 2, NSH], F32R, tag="qt32")        # normalized Q^T (FC stationary)
        wtbuf = sb.tile([P, 4, OUT], F32R, tag="wtbuf")      # W^T feature tiles
        bbuf = sb.tile([1, OUT], F32R, tag="bbuf")           # bias row
        hbuf = sb.tile([P, NT, OUT], F32, tag="hbuf")        # output staging
        ident = sb.tile([P, P], F32, tag="ident")
        ones_row = sb.tile([1, P], F32R, tag="ones_row")
        ones_f32 = sb.tile([1, P], F32, tag="ones_f32")
        n2k = sb.tile([P, MT], F32, tag="n2k")
        nk = sb.tile([P, MT], F32, tag="nk")
        rk = sb.tile([P, MT], F32, tag="rk")                # 1/||k|| per key row
        n2q = sb.tile([P, NT], F32, tag="n2q")
        nq = sb.tile([P, NT], F32, tag="nq")                # ||q|| per query row
        rq = sb.tile([P, NT], F32, tag="rq")                # 1/||q||
        rinv = sb.tile([P, NT], F32, tag="rinv")            # 1/rowsum

        make_identity(nc, ident)
        nc.vector.memset(ones_f32, 1.0)
        nc.vector.tensor_copy(out=ones_row, in_=ones_f32)
        for g in range(4):
            nc.vector.memset(vc[g][:, :, D : D + 2], 0.0)
            nc.vector.memset(vc[g][:, :, D : D + 1], 1.0)

        # ---- DMAs; q first (it gates the Q-norm -> QK chain) ----
        nc.sync.dma_start(qbuf[:], q_d[:].rearrange("(no p) d -> p no d", p=P))
        kt_r = kt_d[:].rearrange("(do p) m -> p do m", p=P)
        v_r = v_d[:].rearrange("(mo p) d -> p mo d", p=P)
        for g in range(4):
            s = slice(g * (M // 4), (g + 1) * (M // 4))
            nc.sync.dma_start(ktc[g][:], kt_r[:, :, s])
            sv = slice(g * (MT // 4), (g + 1) * (MT // 4))
            nc.sync.dma_start(vc[g][:, :, 0:D], v_r[:, sv, :])
        nc.sync.dma_start(wtbuf[:], wt_d[:].rearrange("(fo p) o -> p fo o", p=P).bitcast(F32R))
        nc.sync.dma_start(bbuf[:], b_d[:].bitcast(F32R))

        # ---- query norms + normalized Q^T (the critical-path chain) ----
        for t in range(NT):
            sq = sbw.tile([P, D], F32, tag="sqs")
            nc.scalar.activation(
                sq, qbuf[:, t, :], ACTF.Square, accum_out=n2q[:, t : t + 1]
            )
        nc.vector.tensor_scalar_max(n2q[:], n2q[:], EPS)
        nc.scalar.sqrt(nq[:], n2q[:])
        nc.vector.reciprocal(rq[:], nq[:])
        for t in range(NT):
            qn = sbw.tile([P, D], F32, tag="qn")
            nc.vector.tensor_scalar_mul(qn, qbuf[:, t, :], rq[:, t : t + 1])
            for dt in range(2):
                tp = psA.tile([P, P], F32, tag="sp")
                nc.tensor.transpose(tp, qn[:, dt * P : (dt + 1) * P], ident)
                nc.vector.tensor_copy(
                    out=qtbuf[:, dt, t * P : (t + 1) * P], in_=tp
                )
                nc.vector.tensor_copy(
                    out=qt32[:, dt, t * P : (t + 1) * P], in_=tp
                )

        # ---- key norms: rk = 1/sqrt(max(sum(k^2), eps)), per chunk ----
        for g in range(4):
            cs = slice(g * (MT // 4), (g + 1) * (MT // 4))
            for mm in range(MT // 4):
                m = g * (MT // 4) + mm
                sq = sbw.tile([P, D], F32, tag="sqk")
                nc.scalar.activation(
                    sq, vc[g][:, mm, 0:D], ACTF.Square,
                    accum_out=n2k[:, m : m + 1],
                )
            nc.vector.tensor_scalar_max(n2k[:, cs], n2k[:, cs], EPS)
            nc.scalar.sqrt(nk[:, cs], n2k[:, cs])
            nc.vector.reciprocal(rk[:, cs], nk[:, cs])

        # ---- main flash-attention loop ----
        for nb in range(NBLKS):
            ncols = slice(nb * NBLK, (nb + 1) * NBLK)
            xps = [
                psX.tile([P, VW], F32, tag="xp", name=f"xp_{nb}_{j}")
                for j in range(4)
            ]
            pts = {}
            for m in range(MT + 1):
                if m < MT:
                    sp = psA.tile([P, NBLK], F32, tag="sp")
                    ktg = ktc[m // 8]
                    ms = slice((m % 8) * P, (m % 8 + 1) * P)
                    nc.tensor.matmul(
                        sp, ktg[:, 0, ms], qtbuf[:, 0, ncols],
                        start=True, stop=False,
                    )
                    nc.tensor.matmul(
                        sp, ktg[:, 1, ms], qtbuf[:, 1, ncols],
                        start=False, stop=True,
                    )
                    pt = sbw.tile([P, NBLK], BF16, tag="pt")
                    nc.scalar.activation(pt, sp, ACTF.Exp, scale=rk[:, m : m + 1])
                    pts[m] = pt
                    if debug and nb == 0 and m == 0:
                        nc.sync.dma_start(dbg_pt[:], pt[:])
                if m >= 1:
                    mm = m - 1
                    pt = pts.pop(mm)
                    for j in range(4):
                        nc.tensor.matmul(
                            xps[j],
                            pt[:, j * P : (j + 1) * P],
                            vc[mm // 8][:, mm % 8, :],
                            start=(mm == 0), stop=(mm == MT - 1),
                        )

            # ---- per-tile epilogue: normalize X, transpose, FC, relu ----
            for j in range(4):
                t = nb * 4 + j
                ts_ = slice(t * P, (t + 1) * P)
                nc.vector.reciprocal(rinv[:, t : t + 1], xps[j][:, D : D + 1])
                xn = sbw.tile([P, D], F32, tag="xn")
                nc.vector.tensor_scalar_mul(xn, xps[j][:, 0:D], rinv[:, t : t + 1])
                if debug and t == 0:
                    nc.sync.dma_start(dbg_xn[:], xn[:])
                xnt = sbw.tile([P, 2, P], F32R, tag="xnt")
                for dt in range(2):
                    tp = psA.tile([P, P], F32, tag="sp")
                    nc.tensor.transpose(tp, xn[:, dt * P : (dt + 1) * P], ident)
                    nc.vector.tensor_copy(out=xnt[:, dt, :], in_=tp)

                ap_ = psF.tile([P, OUT], F32, tag="fc")
                bp_ = psF.tile([P, OUT], F32, tag="fc")
                nc.tensor.matmul(
                    ap_, qt32[:, 0, ts_], wtbuf[:, 0, :],
                    start=True, stop=False,
                )
                nc.tensor.matmul(
                    ap_, qt32[:, 1, ts_], wtbuf[:, 1, :],
                    start=False, stop=True,
                )
                nc.tensor.matmul(
                    bp_, xnt[:, 0, :], wtbuf[:, 2, :],
                    start=True, stop=False,
                )
                nc.tensor.matmul(
                    bp_, xnt[:, 1, :], wtbuf[:, 3, :],
                    start=False, stop=False,
                )
                nc.tensor.matmul(
                    bp_, ones_row, bbuf, start=False, stop=True,
                )
                t1 = sbw.tile([P, OUT], F32, tag="t1")
                nc.vector.tensor_scalar_mul(t1, ap_, nq[:, t : t + 1])
                nc.vector.tensor_add(out=hbuf[:, t, :], in0=t1, in1=bp_)
                nc.vector.tensor_scalar_max(hbuf[:, t, :], hbuf[:, t, :], 0.0)

        nc.sync.dma_start(out_d[:].rearrange("(no p) o -> p no o", p=P), hbuf[:])
        if debug:
            nc.sync.dma_start(dbg_qt[:], qtbuf[:])
            nc.sync.dma_start(dbg_rk[:], rk[:])
            nc.sync.dma_start(dbg_rq[:], rq[:])
            nc.sync.dma_start(dbg_ri[:], rinv[:])

    nc.compile()
    return nc


_NC = None


def _get_nc():
    global _NC
    if _NC is None:
        _NC = build_nc()
    return _NC


def _run(inputs, trace=False):
    em1 = np.asarray(inputs["em1"], dtype=np.float32)
    em2 = np.asarray(inputs["em2"], dtype=np.float32)
    W = np.asarray(inputs["W"], dtype=np.float32)
    b = np.asarray(inputs["b"], dtype=np.float32)

    wt = np.ascontiguousarray(W.T)
    brow = np.ascontiguousarray(b[None, :])
    kts = [np.ascontiguousarray(em2[bi].T).astype(NPBF16) for bi in range(B)]
    vs = [em2[bi].astype(NPBF16) for bi in range(B)]
    in_maps = []
    for c in range(8):
        bi, qi = c // 4, c % 4
        in_maps.append(
            {
                "q": np.ascontiguousarray(em1[bi, qi * NSH : (qi + 1) * NSH]),
                "kt": kts[bi],
                "v": vs[bi],
                "wt": wt,
                "bias": brow,
            }
        )

    res = run_bass_kernel_spmd(_get_nc(), in_maps, core_ids=list(range(8)), trace=trace)
    out = np.empty((B, N, OUT), dtype=np.float32)
    for c in range(8):
        bi, qi = c // 4, c % 4
        out[bi, qi * NSH : (qi + 1) * NSH] = res.results[c]["out"]
    return out, res


def kernel(**inputs) -> np.ndarray:
    out, _ = _run(inputs, trace=False)
    return out


# revision 18
# speedup vs baseline: 1.4266x; 1.2072x over previous
"""Trainium2 Bass kernel for the cross-attention layer:

    s   = cosine_sim(em1, em2)          # [B, N, M]
    p   = softmax(s, axis=-1)
    x   = p @ em2                       # [B, N, D]
    out = relu(concat([em1, x]) @ W.T + b)

Sharding: 8 cores, core c = 4*b + i handles batch b, query rows
[i*1024, (i+1)*1024).  em2 is replicated per batch (flash-attention row
sharding).  The score matrix never touches HBM.

Per-core layout choices:
  - All matmul operands are bf16 (separate, pipelined LDWEIGHTS; fp32/
    fp32r matmuls self-load weights and serialize ~176ns per matmul).
    Accumulation stays fp32 in PSUM.
  - QK^T is computed as S^T tiles [m=128, n<=512]: stationary = K^T tile
    (host-pretransposed em2 in bf16), moving = normalized Q^T (built
    on-chip via PE transposes).
  - key norms are folded into the exp() activation's per-partition
    scale, so raw em2 serves as both K^T and V; exp writes bf16 P^T
    tiles that feed the PV matmul directly as stationary weights.
  - V gets a ones-column appended in SBUF; the PV matmul then yields
    [X | rowsum] in one accumulation and X/rowsum is a per-partition
    scalar multiply.
  - The final FC runs off two PSUM accumulations: A = Qnorm^T.T @ W1^T
    (rescaled by per-row ||q|| afterwards, avoiding a transpose of raw
    em1) and B = Xnorm^T.T @ W2^T + bias (ones-row matmul).
  - Norm square-reductions run on VectorE (tensor_tensor_reduce), not
    ScalarE: ScalarE is saturated by the 64 exp() tiles.
"""

import sys

if "/opt/trn_rl_repo" not in sys.path:
    sys.path.insert(0, "/opt/trn_rl_repo")

from contextlib import ExitStack

import numpy as np

import concourse.bass as bass
import concourse.mybir as mybir
import concourse.tile as tile
from concourse import bacc
from concourse.bass_utils import run_bass_kernel_spmd
from concourse.masks import make_identity

B, N, M, D = 2, 4096, 4096, 256
NSH = N // 4          # query rows per core
P = 128
NT = NSH // P         # 8 query tiles per core
MT = M // P           # 32 key tiles
OUT = 512
EPS = 1e-6
F32 = mybir.dt.float32
F32R = mybir.dt.float32r
BF16 = mybir.dt.bfloat16
ACTF = mybir.ActivationFunctionType
ALU = mybir.AluOpType
NPBF16 = mybir.dt.np(BF16)

NBLK = 512            # query columns per S^T block
NBLKS = NSH // NBLK   # 2
VW = D + 2            # V' width: ones col at D, zero pad at D+1


def build_nc(debug=False):
    nc = bacc.Bacc("TRN2", target_bir_lowering=False)
    q_d = nc.declare_dram_parameter("q", [NSH, D], F32, isOutput=False)
    kt_d = nc.declare_dram_parameter("kt", [D, M], BF16, isOutput=False)
    v_d = nc.declare_dram_parameter("v", [M, D], BF16, isOutput=False)
    wt_d = nc.declare_dram_parameter("wt", [D, OUT], F32, isOutput=False)
    wt2_d = nc.declare_dram_parameter("wt2", [D, OUT], BF16, isOutput=False)
    b_d = nc.declare_dram_parameter("bias", [1, OUT], BF16, isOutput=False)
    out_d = nc.declare_dram_parameter("out", [NSH, OUT], F32, isOutput=True)
    if debug:
        dbg_qt = nc.declare_dram_parameter("dbg_qt", [P, 2, NSH], BF16, isOutput=True)
        dbg_rk = nc.declare_dram_parameter("dbg_rk", [P, MT], F32, isOutput=True)
        dbg_rq = nc.declare_dram_parameter("dbg_rq", [P, NT], F32, isOutput=True)
        dbg_pt = nc.declare_dram_parameter("dbg_pt", [P, NBLK], BF16, isOutput=True)
        dbg_xn = nc.declare_dram_parameter("dbg_xn", [P, D], F32, isOutput=True)
        dbg_ri = nc.declare_dram_parameter("dbg_ri", [P, NT], F32, isOutput=True)

    with ExitStack() as ctx:
        tc = ctx.enter_context(tile.TileContext(nc))
        sb = ctx.enter_context(tc.tile_pool(name="sb", bufs=1))
        sbw = ctx.enter_context(tc.tile_pool(name="sbw", bufs=3))
        psA = ctx.enter_context(tc.tile_pool(name="psA", bufs=2, space="PSUM"))
        psX = ctx.enter_context(tc.tile_pool(name="psX", bufs=4, space="PSUM"))
        psF = ctx.enter_context(tc.tile_pool(name="psF", bufs=2, space="PSUM"))

        # ---- persistent SBUF buffers ----
        qbuf = sb.tile([P, NT, D], F32, tag="qbuf")         # raw Q, natural
        ktc = [
            sb.tile([P, 2, M // 4], BF16, tag=f"ktc{g}", name=f"ktc{g}")
            for g in range(4)
        ]
        vc = [
            sb.tile([P, MT // 4, VW], BF16, tag=f"vc{g}", name=f"vc{g}")
            for g in range(4)
        ]
        qtbuf = sb.tile([P, 2, NSH], BF16, tag="qtbuf")      # normalized Q^T (QK moving)
        qt32 = sb.tile([P, 2, NSH], F32R, tag="qt32")        # normalized Q^T (FC stationary)
        wtbufA = sb.tile([P, 2, OUT], F32R, tag="wtbufA")    # W1^T (em1 part, f32r)
        wtbufB = sb.tile([P, 2, OUT], BF16, tag="wtbufB")    # W2^T (x part, bf16)
        bbuf = sb.tile([1, OUT], BF16, tag="bbuf")           # bias row
        hbuf = sb.tile([P, NT, OUT], F32, tag="hbuf")        # output staging
        ident = sb.tile([P, P], F32, tag="ident")
        identb = sb.tile([P, P], BF16, tag="identb")
        ones_row = sb.tile([1, P], BF16, tag="ones_row")
        n2k = sb.tile([P, MT], F32, tag="n2k")
        nk = sb.tile([P, MT], F32, tag="nk")
        rk = sb.tile([P, MT], F32, tag="rk")                # 1/||k|| per key row
        n2q = sb.tile([P, NT], F32, tag="n2q")
        nq = sb.tile([P, NT], F32, tag="nq")                # ||q|| per query row
        rq = sb.tile([P, NT], F32, tag="rq")                # 1/||q||
        rinv = sb.tile([P, NT], F32, tag="rinv")            # 1/rowsum

        make_identity(nc, ident)
        make_identity(nc, identb)
        nc.vector.memset(ones_row, 1.0)
        for g in range(4):
            nc.vector.memset(vc[g][:, :, D : D + 2], 0.0)
            nc.vector.memset(vc[g][:, :, D : D + 1], 1.0)

        # ---- DMAs; q first (it gates the Q-norm -> QK chain) ----
        nc.sync.dma_start(qbuf[:], q_d[:].rearrange("(no p) d -> p no d", p=P))
        kt_r = kt_d[:].rearrange("(do p) m -> p do m", p=P)
        v_r = v_d[:].rearrange("(mo p) d -> p mo d", p=P)
        for g in range(4):
            s = slice(g * (M // 4), (g + 1) * (M // 4))
            nc.sync.dma_start(ktc[g][:], kt_r[:, :, s])
            sv = slice(g * (MT // 4), (g + 1) * (MT // 4))
            nc.sync.dma_start(vc[g][:, :, 0:D], v_r[:, sv, :])
        nc.sync.dma_start(
            wtbufA[:], wt_d[:].rearrange("(fo p) o -> p fo o", p=P).bitcast(F32R)
        )
        nc.sync.dma_start(
            wtbufB[:], wt2_d[:].rearrange("(fo p) o -> p fo o", p=P)
        )
        nc.sync.dma_start(bbuf[:], b_d[:])

        # ---- query norms + normalized Q^T (the critical-path chain) ----
        for t in range(NT):
            sq = sbw.tile([P, D], F32, tag="sqs")
            nc.scalar.activation(
                sq, qbuf[:, t, :], ACTF.Square, accum_out=n2q[:, t : t + 1]
            )
        nc.vector.tensor_scalar_max(n2q[:], n2q[:], EPS)
        nc.scalar.sqrt(nq[:], n2q[:])
        nc.vector.reciprocal(rq[:], nq[:])

        def q_chain(trange):
            for t in trange:
                qn = sbw.tile([P, D], F32, tag="qn", name=f"qn{t}")
                nc.vector.tensor_scalar_mul(qn, qbuf[:, t, :], rq[:, t : t + 1])
                for dt in range(2):
                    tp = psA.tile([P, P], F32, tag="sp", name=f"tq{t}_{dt}")
                    nc.tensor.transpose(tp, qn[:, dt * P : (dt + 1) * P], ident)
                    nc.vector.tensor_copy(
                        out=qtbuf[:, dt, t * P : (t + 1) * P], in_=tp
                    )
                    nc.vector.tensor_copy(
                        out=qt32[:, dt, t * P : (t + 1) * P], in_=tp
                    )

        def k_chain(g):
            # rk = 1/sqrt(max(sum(k^2), eps)); square+reduce on VectorE
            cs = slice(g * (MT // 4), (g + 1) * (MT // 4))
            for mm in range(MT // 4):
                m = g * (MT // 4) + mm
                sq = sbw.tile([P, D], BF16, tag="sqk", name=f"sqk{m}")
                nc.vector.tensor_mul(
                    out=sq, in0=vc[g][:, mm, 0:D], in1=vc[g][:, mm, 0:D]
                )
                nc.vector.tensor_reduce(
                    n2k[:, m : m + 1], sq, mybir.AxisListType.X, ALU.add
                )
            nc.vector.tensor_scalar_max(n2k[:, cs], n2k[:, cs], EPS)
            nc.scalar.sqrt(nk[:, cs], n2k[:, cs])
            nc.vector.reciprocal(rk[:, cs], nk[:, cs])

        q_chain(range(0, 4))     # unblocks QK for n-block 0
        k_chain(0)               # unblocks exp(m=0..7)
        q_chain(range(4, NT))
        for g in range(1, 4):
            k_chain(g)

        # ---- main flash-attention loop ----
        for nb in range(NBLKS):
            ncols = slice(nb * NBLK, (nb + 1) * NBLK)
            xps = [
                psX.tile([P, VW], F32, tag="xp", name=f"xp_{nb}_{j}")
                for j in range(4)
            ]
            pts = {}
            for m in range(MT + 1):
                if m < MT:
                    sp = psA.tile([P, NBLK], F32, tag="sp")
                    ktg = ktc[m // 8]
                    ms = slice((m % 8) * P, (m % 8 + 1) * P)
                    nc.tensor.matmul(
                        sp, ktg[:, 0, ms], qtbuf[:, 0, ncols],
                        start=True, stop=False,
                    )
                    nc.tensor.matmul(
                        sp, ktg[:, 1, ms], qtbuf[:, 1, ncols],
                        start=False, stop=True,
                    )
                    pt = sbw.tile([P, NBLK], BF16, tag="pt")
                    nc.scalar.activation(pt, sp, ACTF.Exp, scale=rk[:, m : m + 1])
                    pts[m] = pt
                    if debug and nb == 0 and m == 0:
                        nc.sync.dma_start(dbg_pt[:], pt[:])
                if m >= 1:
                    mm = m - 1
                    pt = pts.pop(mm)
                    for j in range(4):
                        nc.tensor.matmul(
                            xps[j],
                            pt[:, j * P : (j + 1) * P],
                            vc[mm // 8][:, mm % 8, :],
                            start=(mm == 0), stop=(mm == MT - 1),
                        )

            # ---- per-tile epilogue: normalize X, transpose, FC, relu ----
            for j in range(4):
                t = nb * 4 + j
                ts_ = slice(t * P, (t + 1) * P)
                nc.vector.reciprocal(rinv[:, t : t + 1], xps[j][:, D : D + 1])
                xn = sbw.tile([P, D], BF16, tag="xn")
                nc.vector.tensor_scalar_mul(xn, xps[j][:, 0:D], rinv[:, t : t + 1])
                if debug and t == 0:
                    nc.sync.dma_start(dbg_xn[:], xn[:])
                xnt = sbw.tile([P, 2, P], BF16, tag="xnt")
                for dt in range(2):
                    tp = psA.tile([P, P], BF16, tag="sp")
                    nc.tensor.transpose(tp, xn[:, dt * P : (dt + 1) * P], identb)
                    nc.vector.tensor_copy(out=xnt[:, dt, :], in_=tp)

                ap_ = psF.tile([P, OUT], F32, tag="fc")
                bp_ = psF.tile([P, OUT], F32, tag="fc")
                nc.tensor.matmul(
                    ap_, qt32[:, 0, ts_], wtbufA[:, 0, :],
                    start=True, stop=False,
                )
                nc.tensor.matmul(
                    ap_, qt32[:, 1, ts_], wtbufA[:, 1, :],
                    start=False, stop=True,
                )
                nc.tensor.matmul(
                    bp_, xnt[:, 0, :], wtbufB[:, 0, :],
                    start=True, stop=False,
                )
                nc.tensor.matmul(
                    bp_, xnt[:, 1, :], wtbufB[:, 1, :],
                    start=False, stop=False,
                )
                nc.tensor.matmul(
                    bp_, ones_row, bbuf, start=False, stop=True,
                )
                t1 = sbw.tile([P, OUT], F32, tag="t1")
                nc.vector.tensor_scalar_mul(t1, ap_, nq[:, t : t + 1])
                nc.vector.tensor_add(out=hbuf[:, t, :], in0=t1, in1=bp_)
                nc.vector.tensor_scalar_max(hbuf[:, t, :], hbuf[:, t, :], 0.0)

        nc.sync.dma_start(out_d[:].rearrange("(no p) o -> p no o", p=P), hbuf[:])
        if debug:
            nc.sync.dma_start(dbg_qt[:], qtbuf[:])
            nc.sync.dma_start(dbg_rk[:], rk[:])
            nc.sync.dma_start(dbg_rq[:], rq[:])
            nc.sync.dma_start(dbg_ri[:], rinv[:])

    nc.compile()
    return nc


_NC = None


def _get_nc():
    global _NC
    if _NC is None:
        _NC = build_nc()
    return _NC


def _run(inputs, trace=False):
    em1 = np.asarray(inputs["em1"], dtype=np.float32)
    em2 = np.asarray(inputs["em2"], dtype=np.float32)
    W = np.asarray(inputs["W"], dtype=np.float32)
    b = np.asarray(inputs["b"], dtype=np.float32)

    wt1 = np.ascontiguousarray(W.T[0:D])
    wt2 = np.ascontiguousarray(W.T[D : 2 * D]).astype(NPBF16)
    brow = np.ascontiguousarray(b[None, :]).astype(NPBF16)
    kts = [np.ascontiguousarray(em2[bi].T).astype(NPBF16) for bi in range(B)]
    vs = [em2[bi].astype(NPBF16) for bi in range(B)]
    in_maps = []
    for c in range(8):
        bi, qi = c // 4, c % 4
        in_maps.append(
            {
                "q": np.ascontiguousarray(em1[bi, qi * NSH : (qi + 1) * NSH]),
                "kt": kts[bi],
                "v": vs[bi],
                "wt": wt1,
                "wt2": wt2,
                "bias": brow,
            }
        )

    res = run_bass_kernel_spmd(_get_nc(), in_maps, core_ids=list(range(8)), trace=trace)
    out = np.empty((B, N, OUT), dtype=np.float32)
    for c in range(8):
        bi, qi = c // 4, c % 4
        out[bi, qi * NSH : (qi + 1) * NSH] = res.results[c]["out"]
    return out, res


def kernel(**inputs) -> np.ndarray:
    out, _ = _run(inputs, trace=False)
    return out
